# revision 2
# baseline (speedup 1.0000x reference)
"""nn_DiTBlock Trainium2 kernel v2: 8-core sharded AdaLN-Zero DiT block.

Sharding: 8 cores = 4 batch elements x 2 query-halves. Each core receives
its batch element's tokens rolled so its own 1024 query tokens come first,
computes K/V over all 2048 tokens, and attention/FFN/output for its own
1024 query rows. No collectives; per-core outputs are disjoint row blocks.

v2 vs v1: all matmul operands in bf16 (weights shipped bf16 from host,
halving HBM traffic); K^T/Q^T stay resident in SBUF (no DRAM roundtrip);
FFN second matmul accumulates over the full d_ff in PSUM with token-major
output (no transpose pass, no SBUF accumulation); residual path stays f32.
"""
import sys
sys.path.insert(0, "/opt/trn_rl_repo")

import numpy as np

from contextlib import ExitStack

import concourse.bass as bass
import concourse.tile as tile
from concourse import mybir
from concourse.masks import make_identity

F32 = mybir.dt.float32
BF16 = mybir.dt.bfloat16
AF = mybir.ActivationFunctionType
ALU = mybir.AluOpType

P = 128
EPS = 1e-5


def ap2(handle, offset, ap):
    return bass.AP(tensor=handle, offset=offset, ap=[list(p) for p in ap])


def build_dit(nc, D=1024, NH=16, DFF=4096, NT=2048, NQ=1024, GELU_FUNC=AF.Gelu):
    HD = 64
    assert NH * HD == D
    DC = D // P            # feature chunks of d_model (8)
    KT = NT // P           # kv token tiles (16)
    QT = NQ // P           # query token tiles (8)
    FC = DFF // P          # d_ff chunks (32)
    FB = 8                 # d_ff groups of 4 chunks for W1 streaming
    FCB = FC // FB         # 4
    HC = HD + 1            # head cols in V_aug (64 data + 1 ones)

    xb = nc.dram_tensor("xb", [NT, D], F32, kind="ExternalInput")
    cb = nc.dram_tensor("cb", [1, D], BF16, kind="ExternalInput")
    W_ada = nc.dram_tensor("W_ada", [D, 6 * D], BF16, kind="ExternalInput")
    b_ada = nc.dram_tensor("b_ada", [1, 6 * D], F32, kind="ExternalInput")
    Wq = nc.dram_tensor("Wq", [D, D], BF16, kind="ExternalInput")
    bq = nc.dram_tensor("bq", [1, D], F32, kind="ExternalInput")
    Wk = nc.dram_tensor("Wk", [D, D], BF16, kind="ExternalInput")
    bk = nc.dram_tensor("bk", [1, D], F32, kind="ExternalInput")
    Wv = nc.dram_tensor("Wv", [D, D], BF16, kind="ExternalInput")
    bv = nc.dram_tensor("bv", [1, D], F32, kind="ExternalInput")
    Wo = nc.dram_tensor("Wo", [D, D], BF16, kind="ExternalInput")
    bo = nc.dram_tensor("bo", [1, D], F32, kind="ExternalInput")
    W1 = nc.dram_tensor("W1", [D, DFF], BF16, kind="ExternalInput")
    b1 = nc.dram_tensor("b1", [1, DFF], F32, kind="ExternalInput")
    W2 = nc.dram_tensor("W2", [DFF, D], BF16, kind="ExternalInput")
    b2 = nc.dram_tensor("b2", [1, D], F32, kind="ExternalInput")
    out = nc.dram_tensor("out", [NQ, D], F32, kind="ExternalOutput")

    with tile.TileContext(nc) as tc, ExitStack() as ctx:
        dram = ctx.enter_context(tc.tile_pool(name="dram", bufs=1, space="DRAM"))
        ada_dram = dram.tile([1, 6 * D], F32)
        adh = ada_dram.tensor

        consts = ctx.enter_context(tc.tile_pool(name="consts", bufs=1))
        ident = consts.tile([P, P], BF16)
        make_identity(nc, ident[:])
        # packed per-partition constants: [eps, one, bk(8), bq(8), b1(32),
        # s1(8), sh1(8), s2(8), sh2(8)] (f32)
        pack = consts.tile([P, 96], F32)
        eps_t = pack[:, 0:1]
        nc.vector.memset(eps_t, EPS)
        one_col = pack[:, 1:2]
        nc.vector.memset(one_col, 1.0)
        bk_pp = pack[:, 2:2 + DC]
        nc.sync.dma_start(bk_pp, ap2(bk, 0, [[1, P], [P, DC]]))
        bq_pp = pack[:, 10:10 + DC]
        nc.sync.dma_start(bq_pp, ap2(bq, 0, [[1, P], [P, DC]]))
        b1_pp = pack[:, 18:18 + FC]
        nc.sync.dma_start(b1_pp, ap2(b1, 0, [[1, P], [P, FC]]))

        _pp_next = [18 + FC]

        def load_pp(off, plus1=False):
            """ada slice as per-partition chunked [P, DC] into pack cols."""
            c0 = _pp_next[0]
            assert c0 + DC <= 96
            t = pack[:, c0:c0 + DC]
            _pp_next[0] = c0 + DC
            nc.sync.dma_start(t, ap2(adh, off, [[1, P], [P, DC]]))
            if plus1:
                nc.vector.tensor_scalar(t, t, scalar1=one_col,
                                        scalar2=None, op0=ALU.add)
            return t

        def load_bc(pool, name, dram_handle, off, n=None):
            n = D if n is None else n
            t = pool.tile([P, n], F32, name=name)
            nc.sync.dma_start(t[:], ap2(dram_handle, off, [[0, P], [1, n]]))
            return t

        def ln_normalize(pool, xt):
            """token-major LN (no affine): (x - mean) * rsqrt(var + eps).
            Returns a BF16 normalized tile."""
            lp = pool.tile([P, 16], F32, tag="lnp")
            stats = lp[:, 0:12].rearrange("p (s f) -> p s f", f=6)
            xv = xt[:].rearrange("p (s f) -> p s f", f=512)
            for s in range(D // 512):
                nc.vector.bn_stats(stats[:, s, :], xv[:, s, :])
            mv = lp[:, 12:14]
            nc.vector.bn_aggr(mv, lp[:, 0:12].rearrange(
                "p (s f) -> p s f", f=6)[:, :D // 512, :])
            sd = lp[:, 14:15]
            nc.scalar.activation(sd, mv[:, 1:2], AF.Sqrt, bias=eps_t)
            rstd = lp[:, 15:16]
            nc.vector.reciprocal(rstd, sd)
            xn = pool.tile([P, D], BF16, tag="xn")
            nc.vector.tensor_scalar(xn[:], xt[:], scalar1=mv[:, 0:1],
                                    scalar2=rstd,
                                    op0=ALU.subtract, op1=ALU.mult)
            return xn

        # ============ Phase A: ada = cb @ W_ada + b_ada -> ada_dram
        with tc.tile_pool(name="ada_w", bufs=3) as awp, \
             tc.tile_pool(name="ada_sb", bufs=3) as asb, \
             tc.tile_pool(name="ada_ps", bufs=2, space="PSUM") as aps:
            cT = asb.tile([P, DC], BF16)
            nc.sync.dma_start(cT[:], ap2(cb, 0, [[1, P], [P, DC]]))
            for j in range(6 * D // 512):
                ps = aps.tile([1, 512], F32, tag="ps")
                for kc in range(DC):
                    wt = awp.tile([P, 512], BF16, tag="w")
                    enga = nc.scalar if kc % 2 == 0 else nc.sync
                    enga.dma_start(
                        wt[:], ap2(W_ada, kc * P * 6 * D + j * 512,
                                   [[6 * D, P], [1, 512]]))
                    nc.tensor.matmul(ps[:], cT[:, kc:kc + 1], wt[:],
                                     start=(kc == 0), stop=(kc == DC - 1))
                bt = asb.tile([1, 512], F32, tag="b")
                nc.sync.dma_start(bt[:], ap2(b_ada, j * 512, [[512, 1], [1, 512]]))
                st = asb.tile([1, 512], F32, tag="s")
                nc.vector.tensor_tensor(st[:], ps[:], bt[:], op=ALU.add)
                nc.sync.dma_start(ap2(adh, j * 512, [[512, 1], [1, 512]]), st[:])

        # SBUF-resident intermediates. Released manually; allocation order is
        # chosen so releases pop in LIFO order per side:
        #   left: consts(ctx) < modg(ctx) < ores(til end of E)
        #         < kqres,vres(til end of D) < phase with-pools
        modg = ctx.enter_context(tc.tile_pool(name="modg", bufs=1))
        ores = tc.alloc_tile_pool(name="ores", bufs=1)
        oT = ores.tile([P, DC, NQ], BF16)
        kqres = tc.alloc_tile_pool(name="kqres", bufs=1)
        vres = tc.alloc_tile_pool(name="vres", bufs=1)
        if True:
            kT = kqres.tile([P, DC, NT], BF16)
            qT = kqres.tile([P, DC, NQ], BF16)
            V_aug = vres.tile([P, KT, NH * HC], BF16)
            with tc.tile_pool(name="hres_pool", bufs=1) as hres_pool, \
                 tc.tile_pool(name="mod1", bufs=1) as mod1, \
                 tc.tile_pool(name="ln1", bufs=3) as lnp, \
                 tc.tile_pool(name="wkq", bufs=24) as wkq, \
                 tc.tile_pool(name="wv", bufs=9) as wvp, \
                 tc.tile_pool(name="tps", bufs=4, space="PSUM") as tps, \
                 tc.tile_pool(name="kqps", bufs=2, space="PSUM") as kqps, \
                 tc.tile_pool(name="vps", bufs=2, space="PSUM") as vps:
                hres = hres_pool.tile([P, DC, NT], BF16)
                # ======== Phase C-LN: LN1 + transpose + fused modulation
                s1_pp = load_pp(1 * D, plus1=True)
                sh1_pp = load_pp(0 * D)
                for t in range(KT):
                    xt = lnp.tile([P, D], F32, tag="x")
                    nc.sync.dma_start(xt[:], ap2(xb, t * P * D,
                                                 [[D, P], [1, D]]))
                    xn = ln_normalize(lnp, xt)
                    for dc in range(DC):
                        pt = tps.tile([P, P], BF16, tag="t")
                        nc.tensor.transpose(pt[:], xn[:, dc * P:(dc + 1) * P],
                                            ident[:])
                        if dc % 2 == 0:
                            nc.vector.tensor_scalar(
                                hres[:, dc, t * P:(t + 1) * P], pt[:],
                                scalar1=s1_pp[:, dc:dc + 1],
                                scalar2=sh1_pp[:, dc:dc + 1],
                                op0=ALU.mult, op1=ALU.add)
                        else:
                            nc.scalar.activation(
                                hres[:, dc, t * P:(t + 1) * P], pt[:],
                                AF.Identity,
                                scale=s1_pp[:, dc:dc + 1],
                                bias=sh1_pp[:, dc:dc + 1])

                # ======== Phase C-KQ: K^T, Q^T -> SBUF (kT/qT)
                for mc in range(DC):
                    for W_, b_pp, is_q in ((Wk, bk_pp, False),
                                           (Wq, bq_pp, True)):
                        ncols = NQ if is_q else NT
                        dst3 = qT if is_q else kT
                        wcol = []
                        for kc in range(DC):
                            wt = wkq.tile([P, P], BF16, tag="w")
                            eng = nc.scalar if kc % 2 == 0 else nc.sync
                            eng.dma_start(
                                wt[:], ap2(W_, kc * P * D + mc * P,
                                           [[D, P], [1, P]]))
                            wcol.append(wt)
                        for ns in range(ncols // 512):
                            ps = kqps.tile([P, 512], F32, tag="ps")
                            for kc in range(DC):
                                nc.tensor.matmul(
                                    ps[:], wcol[kc][:],
                                    hres[:, kc, ns * 512:(ns + 1) * 512],
                                    start=(kc == 0), stop=(kc == DC - 1))
                            nc.vector.tensor_scalar(
                                dst3[:, mc, ns * 512:(ns + 1) * 512], ps[:],
                                scalar1=b_pp[:, mc:mc + 1],
                                scalar2=None, op0=ALU.add)

                # ======== Phase C-V: V_aug (token-major, per head + ones col)
                bv_bc = load_bc(mod1, "bv_bc", bv, 0)
                nc.scalar.copy(
                    V_aug[:].rearrange("p t (h c) -> p t h c",
                                       c=HC)[:, :, :, HD:HD + 1],
                    one_col.to_broadcast((P, KT, NH, 1)))
                for nh in range(D // 512):
                    wvt = []
                    for kc in range(DC):
                        wt = wvp.tile([P, 512], BF16, tag="wv")
                        eng = nc.scalar if kc % 2 == 0 else nc.sync
                        eng.dma_start(
                            wt[:], ap2(Wv, kc * P * D + nh * 512,
                                       [[D, P], [1, 512]]))
                        wvt.append(wt)
                    for t in range(KT):
                        ps = vps.tile([P, 512], F32, tag="ps")
                        for kc in range(DC):
                            nc.tensor.matmul(
                                ps[:], hres[:, kc, t * P:(t + 1) * P],
                                wvt[kc][:],
                                start=(kc == 0), stop=(kc == DC - 1))
                        dst = V_aug[:, t,
                                    nh * 8 * HC:(nh + 1) * 8 * HC].rearrange(
                            "p (h c) -> p h c", c=HC)[:, :, 0:HD]
                        nc.vector.tensor_tensor(
                            dst, ps[:].rearrange("p (h c) -> p h c", c=HD),
                            bv_bc[:, nh * 512:(nh + 1) * 512].rearrange(
                                "p (h c) -> p h c", c=HD),
                            op=ALU.add)

            # ============ Phase D: attention per head
            with tc.tile_pool(name="expool", bufs=3) as expool, \
                 tc.tile_pool(name="rzp", bufs=2) as rzp, \
                 tc.tile_pool(name="sps", bufs=2, space="PSUM") as sps, \
                 tc.tile_pool(name="ops", bufs=2, space="PSUM") as ops:
                for h in range(NH):
                    hcc, hr = h // 2, (h % 2) * HD
                    po = ops.tile([HC, NQ], F32, tag="o")
                    for kt in range(KT):
                        pss = sps.tile([P, NQ], F32, tag="s")
                        for qs in range(NQ // 512):
                            nc.tensor.matmul(
                                pss[:, qs * 512:(qs + 1) * 512],
                                kT[hr:hr + HD, hcc, kt * P:(kt + 1) * P],
                                qT[hr:hr + HD, hcc, qs * 512:(qs + 1) * 512],
                                start=True, stop=True)
                        ex = expool.tile([P, NQ], BF16, tag="ex")
                        nc.scalar.activation(ex[:], pss[:], AF.Exp, scale=0.125)
                        for qs in range(NQ // 512):
                            nc.tensor.matmul(
                                po[:, qs * 512:(qs + 1) * 512],
                                V_aug[:, kt, h * HC:(h + 1) * HC],
                                ex[:, qs * 512:(qs + 1) * 512],
                                start=(kt == 0), stop=(kt == KT - 1))
                    rz = rzp.tile([1, NQ], F32, tag="rz")
                    nc.vector.reciprocal(rz[:], po[HD:HD + 1, :])
                    rzb = rzp.tile([HD, NQ], F32, tag="rzb")
                    nc.gpsimd.partition_broadcast(rzb[:], rz[:])
                    nc.vector.tensor_tensor(oT[hr:hr + HD, hcc, :],
                                            po[0:HD, :], rzb[:], op=ALU.mult)
            vres.release()
            kqres.release()

        # ============ Phase E: out-proj (gate1 folded into Wo) + residual
        # + LN2 (token-major x2 kept for the final residual)
        res2 = ctx.enter_context(tc.tile_pool(name="res2", bufs=1, side="right"))
        h2T = res2.tile([P, DC, NQ], BF16)
        x2 = res2.tile([P, QT, D], F32)
        g2_bc = load_bc(modg, "g2_bc", adh, 5 * D)
        with tc.tile_pool(name="mod2", bufs=1) as mod2, \
             tc.tile_pool(name="wo", bufs=1) as wop, \
             tc.tile_pool(name="ln2", bufs=3) as ln2p, \
             tc.tile_pool(name="aops", bufs=2, space="PSUM") as aops, \
             tc.tile_pool(name="tps2", bufs=4, space="PSUM") as tps2:
            g1_bc = load_bc(mod2, "g1_bc", adh, 2 * D)
            g1_bcb = mod2.tile([P, D], BF16, name="g1_bcb")
            nc.vector.tensor_copy(g1_bcb[:], g1_bc[:])
            s2_pp = load_pp(4 * D, plus1=True)
            sh2_pp = load_pp(3 * D)
            # cst_bc = g1*bo + g2*b2 (broadcast row; pre-folded into x2)
            cst_bc = mod2.tile([P, D], F32, name="cst_bc")
            tmp1 = ln2p.tile([P, D], F32, tag="tmp1", bufs=1)
            nc.sync.dma_start(tmp1[:], ap2(bo, 0, [[0, P], [1, D]]))
            nc.vector.tensor_tensor(cst_bc[:], g1_bc[:], tmp1[:], op=ALU.mult)
            nc.sync.dma_start(tmp1[:], ap2(b2, 0, [[0, P], [1, D]]))
            nc.vector.tensor_tensor(tmp1[:], tmp1[:], g2_bc[:], op=ALU.mult)
            nc.vector.tensor_tensor(cst_bc[:], cst_bc[:], tmp1[:], op=ALU.add)
            # Wo' rhs tiles (g1 pre-scaled), all resident, read once
            wot = {}
            for nh in range(D // 512):
                for oc in range(DC):
                    wt = wop.tile([P, 512], BF16, tag=f"w{nh}_{oc}")
                    eng = nc.scalar if oc % 2 == 0 else nc.sync
                    eng.dma_start(
                        wt[:], ap2(Wo, oc * P * D + nh * 512,
                                   [[D, P], [1, 512]]))
                    nc.vector.tensor_tensor(
                        wt[:], wt[:], g1_bcb[:, nh * 512:(nh + 1) * 512],
                        op=ALU.mult)
                    wot[(nh, oc)] = wt
            for t in range(QT):
                xt = ln2p.tile([P, D], F32, tag="x")
                nc.sync.dma_start(xt[:], ap2(xb, t * P * D, [[D, P], [1, D]]))
                nc.vector.tensor_tensor(xt[:], xt[:], cst_bc[:], op=ALU.add)
                for nh in range(D // 512):
                    ps = aops.tile([P, 512], F32, tag="ps")
                    for oc in range(DC):
                        nc.tensor.matmul(ps[:], oT[:, oc, t * P:(t + 1) * P],
                                         wot[(nh, oc)][:],
                                         start=(oc == 0), stop=(oc == DC - 1))
                    sl = slice(nh * 512, (nh + 1) * 512)
                    nc.vector.tensor_tensor(x2[:, t, sl], xt[:, sl], ps[:],
                                            op=ALU.add)
                xn2 = ln_normalize(ln2p, x2[:, t, :])
                for dc in range(DC):
                    pt = tps2.tile([P, P], BF16, tag="t")
                    nc.tensor.transpose(pt[:], xn2[:, dc * P:(dc + 1) * P],
                                        ident[:])
                    if dc % 2 == 0:
                        nc.vector.tensor_scalar(
                            h2T[:, dc, t * P:(t + 1) * P], pt[:],
                            scalar1=s2_pp[:, dc:dc + 1],
                            scalar2=sh2_pp[:, dc:dc + 1],
                            op0=ALU.mult, op1=ALU.add)
                    else:
                        nc.scalar.activation(
                            h2T[:, dc, t * P:(t + 1) * P], pt[:],
                            AF.Identity,
                            scale=s2_pp[:, dc:dc + 1],
                            bias=sh2_pp[:, dc:dc + 1])
        ores.release()

        # ============ Phase F: FFN. g = gelu(h2 @ W1 + b1) staged in SBUF
        # (feature-major); second matmul accumulates all of d_ff in PSUM with
        # token-major output; final residual+gate fused per token tile.
        with tc.tile_pool(name="w1p", bufs=2) as w1p, \
             tc.tile_pool(name="w2p", bufs=1) as w2p, \
             tc.tile_pool(name="gres", bufs=1) as gres, \
             tc.tile_pool(name="fin", bufs=2) as finp, \
             tc.tile_pool(name="gps", bufs=2, space="PSUM") as gps, \
             tc.tile_pool(name="fps", bufs=2, space="PSUM") as fps:
            g = gres.tile([P, FC, NQ], BF16)
            w2t = []
            for f in range(FC):
                wt = w2p.tile([P, D], BF16, tag=f"w2_{f}")
                eng = nc.sync if f % 2 == 0 else nc.scalar
                eng.dma_start(wt[:], ap2(W2, f * P * D, [[D, P], [1, D]]))
                w2t.append(wt)
            for fb in range(FB):
                w1t = []
                for kc in range(DC):
                    wt = w1p.tile([P, FCB * P], BF16, tag=f"w1_{kc}")
                    eng1 = nc.scalar if kc % 2 == 0 else nc.sync
                    eng1.dma_start(
                        wt[:], ap2(W1, kc * P * DFF + fb * FCB * P,
                                   [[DFF, P], [1, FCB * P]]))
                    w1t.append(wt)
                for fc in range(FCB):
                    f = fb * FCB + fc
                    for qs in range(NQ // 512):
                        psg = gps.tile([P, 512], F32, tag="g")
                        for kc in range(DC):
                            nc.tensor.matmul(
                                psg[:], w1t[kc][:, fc * P:(fc + 1) * P],
                                h2T[:, kc, qs * 512:(qs + 1) * 512],
                                start=(kc == 0), stop=(kc == DC - 1))
                        if GELU_FUNC == "sigmoid_approx":
                            # CoreSim has no Gelu; x*sigmoid(1.702x) stand-in
                            xb1 = finp.tile([P, 512], F32, tag="xb1")
                            nc.vector.tensor_scalar(xb1[:], psg[:],
                                                    scalar1=b1_pp[:, f:f + 1],
                                                    scalar2=None, op0=ALU.add)
                            sg = finp.tile([P, 512], F32, tag="sg")
                            nc.scalar.activation(sg[:], xb1[:], AF.Sigmoid,
                                                 scale=1.702)
                            nc.vector.tensor_tensor(
                                g[:, f, qs * 512:(qs + 1) * 512],
                                xb1[:], sg[:], op=ALU.mult)
                        else:
                            nc.scalar.activation(
                                g[:, f, qs * 512:(qs + 1) * 512], psg[:],
                                GELU_FUNC, bias=b1_pp[:, f:f + 1])
            for t in range(QT):
                psf = fps.tile([P, D], F32, tag="f")
                for f in range(FC):
                    for nh in range(D // 512):
                        nc.tensor.matmul(
                            psf[:, nh * 512:(nh + 1) * 512],
                            g[:, f, t * P:(t + 1) * P],
                            w2t[f][:, nh * 512:(nh + 1) * 512],
                            start=(f == 0), stop=(f == FC - 1))
                o_t = finp.tile([P, D], F32, tag="o")
                nc.vector.tensor_tensor(o_t[:], psf[:], g2_bc[:], op=ALU.mult)
                nc.vector.tensor_tensor(o_t[:], o_t[:], x2[:, t, :], op=ALU.add)
                nc.sync.dma_start(ap2(out, t * P * D, [[D, P], [1, D]]), o_t[:])

    return {"ada": ada_dram.tensor.name}


_COMPILED = None


def _get_compiled():
    global _COMPILED
    if _COMPILED is None:
        from concourse import bacc
        nc = bacc.Bacc("TRN2", target_bir_lowering=False, debug=False)
        build_dit(nc)
        nc.compile()
        _COMPILED = nc
    return _COMPILED


def _to_bf16(a):
    import ml_dtypes
    return np.asarray(a, np.float32).astype(ml_dtypes.bfloat16)


def make_in_maps(x, c, W_ada, b_ada, Wq, bq, Wk, bk, Wv, bv, Wo, bo,
                 W1, b1, W2, b2):
    x = np.ascontiguousarray(np.asarray(x, dtype=np.float32))
    shared = {
        "W_ada": _to_bf16(W_ada),
        "b_ada": np.asarray(b_ada, np.float32).reshape(1, -1),
        "Wq": _to_bf16(Wq), "bq": np.asarray(bq, np.float32).reshape(1, -1),
        "Wk": _to_bf16(Wk), "bk": np.asarray(bk, np.float32).reshape(1, -1),
        "Wv": _to_bf16(Wv), "bv": np.asarray(bv, np.float32).reshape(1, -1),
        "Wo": _to_bf16(Wo), "bo": np.asarray(bo, np.float32).reshape(1, -1),
        "W1": _to_bf16(W1), "b1": np.asarray(b1, np.float32).reshape(1, -1),
        "W2": _to_bf16(W2), "b2": np.asarray(b2, np.float32).reshape(1, -1),
    }
    cb = _to_bf16(np.asarray(c, np.float32))
    in_maps = []
    for core in range(8):
        b, s = core // 2, core % 2
        xb_ = np.roll(x[b], -1024 * s, axis=0) if s else x[b]
        m = dict(shared)
        m["xb"] = np.ascontiguousarray(xb_)
        m["cb"] = np.ascontiguousarray(cb[b:b + 1])
        in_maps.append(m)
    return in_maps


def kernel(x, c, W_ada, b_ada, Wq, bq, Wk, bk, Wv, bv, Wo, bo, W1, b1, W2, b2):
    from concourse import bass_utils
    nc = _get_compiled()
    B, N, D = x.shape
    assert (B, N, D) == (4, 2048, 1024)
    in_maps = make_in_maps(x, c, W_ada, b_ada, Wq, bq, Wk, bk, Wv, bv,
                           Wo, bo, W1, b1, W2, b2)

    last_err = None
    for _attempt in range(3):
        try:
            res = bass_utils.run_bass_kernel_spmd(nc, in_maps, core_ids=list(range(8)))
            break
        except Exception as e:  # transient NRT device errors; retry
            last_err = e
    else:
        raise last_err

    out = np.empty((4, 2048, 1024), np.float32)
    for core in range(8):
        b, s = core // 2, core % 2
        out[b, s * 1024:(s + 1) * 1024, :] = res.results[core]["out"]
    return out


# revision 3
# speedup vs baseline: 1.6686x; 1.6686x over previous
"""nn_DiTBlock Trainium2 kernel v2: 8-core sharded AdaLN-Zero DiT block.

Sharding: 8 cores = 4 batch elements x 2 query-halves. Each core receives
its batch element's tokens rolled so its own 1024 query tokens come first,
computes K/V over all 2048 tokens, and attention/FFN/output for its own
1024 query rows. No collectives; per-core outputs are disjoint row blocks.

v2 vs v1: all matmul operands in bf16 (weights shipped bf16 from host,
halving HBM traffic); K^T/Q^T stay resident in SBUF (no DRAM roundtrip);
FFN second matmul accumulates over the full d_ff in PSUM with token-major
output (no transpose pass, no SBUF accumulation); residual path stays f32.
"""
import sys
sys.path.insert(0, "/opt/trn_rl_repo")

import numpy as np

from contextlib import ExitStack

import concourse.bass as bass
import concourse.tile as tile
from concourse import mybir
from concourse.masks import make_identity

F32 = mybir.dt.float32
BF16 = mybir.dt.bfloat16
AF = mybir.ActivationFunctionType
ALU = mybir.AluOpType

P = 128
EPS = 1e-5


def ap2(handle, offset, ap):
    return bass.AP(tensor=handle, offset=offset, ap=[list(p) for p in ap])


def build_dit(nc, D=1024, NH=16, DFF=4096, NT=2048, NQ=1024, GELU_FUNC=AF.Gelu):
    HD = 64
    assert NH * HD == D
    DC = D // P            # feature chunks of d_model (8)
    KT = NT // P           # kv token tiles (16)
    QT = NQ // P           # query token tiles (8)
    FC = DFF // P          # d_ff chunks (32)
    FB = 8                 # d_ff groups of 4 chunks for W1 streaming
    FCB = FC // FB         # 4
    HC = HD + 1            # head cols in V_aug (64 data + 1 ones)

    xb = nc.dram_tensor("xb", [NT, D], F32, kind="ExternalInput")
    cb = nc.dram_tensor("cb", [1, D], BF16, kind="ExternalInput")
    W_ada = nc.dram_tensor("W_ada", [D, 6 * D], BF16, kind="ExternalInput")
    b_ada = nc.dram_tensor("b_ada", [1, 6 * D], F32, kind="ExternalInput")
    Wq = nc.dram_tensor("Wq", [D, D], BF16, kind="ExternalInput")
    bq = nc.dram_tensor("bq", [1, D], F32, kind="ExternalInput")
    Wk = nc.dram_tensor("Wk", [D, D], BF16, kind="ExternalInput")
    bk = nc.dram_tensor("bk", [1, D], F32, kind="ExternalInput")
    Wv = nc.dram_tensor("Wv", [D, D], BF16, kind="ExternalInput")
    bv = nc.dram_tensor("bv", [1, D], F32, kind="ExternalInput")
    Wo = nc.dram_tensor("Wo", [D, D], BF16, kind="ExternalInput")
    bo = nc.dram_tensor("bo", [1, D], F32, kind="ExternalInput")
    W1 = nc.dram_tensor("W1", [D, DFF], BF16, kind="ExternalInput")
    b1 = nc.dram_tensor("b1", [1, DFF], F32, kind="ExternalInput")
    W2 = nc.dram_tensor("W2", [DFF, D], BF16, kind="ExternalInput")
    b2 = nc.dram_tensor("b2", [1, D], F32, kind="ExternalInput")
    out = nc.dram_tensor("out", [NQ, D], F32, kind="ExternalOutput")

    with tile.TileContext(nc) as tc, ExitStack() as ctx:
        dram = ctx.enter_context(tc.tile_pool(name="dram", bufs=1, space="DRAM"))
        # one DRAM tile per 512-wide ada chunk so consumers only wait for
        # the chunks they read (whole-tile deps would serialize on all 12)
        ada_chunks = [dram.tile([1, 512], F32, name=f"ada{j}")
                      for j in range(12)]

        consts = ctx.enter_context(tc.tile_pool(name="consts", bufs=1))
        ident = consts.tile([P, P], BF16)
        make_identity(nc, ident[:])
        # packed per-partition constants: [eps, one, bk(8), bq(8), b1(32),
        # s1(8), sh1(8), s2(8), sh2(8)] (f32)
        pack = consts.tile([P, 96], F32)
        eps_t = pack[:, 0:1]
        nc.vector.memset(eps_t, EPS)
        one_col = pack[:, 1:2]
        nc.vector.memset(one_col, 1.0)
        bk_pp = pack[:, 2:2 + DC]
        nc.sync.dma_start(bk_pp, ap2(bk, 0, [[1, P], [P, DC]]))
        bq_pp = pack[:, 10:10 + DC]
        nc.sync.dma_start(bq_pp, ap2(bq, 0, [[1, P], [P, DC]]))
        b1_pp = pack[:, 18:18 + FC]
        nc.sync.dma_start(b1_pp, ap2(b1, 0, [[1, P], [P, FC]]))

        _pp_next = [18 + FC]

        def load_pp(off, plus1=False):
            """ada slice as per-partition chunked [P, DC] into pack cols."""
            c0 = _pp_next[0]
            assert c0 + DC <= 96
            assert off % 512 == 0
            t = pack[:, c0:c0 + DC]
            _pp_next[0] = c0 + DC
            for half in range(2):
                ch = ada_chunks[off // 512 + half]
                nc.sync.dma_start(
                    t[:, half * 4:(half + 1) * 4],
                    ap2(ch.tensor, 0, [[1, P], [P, 4]]))
            if plus1:
                nc.vector.tensor_scalar(t, t, scalar1=one_col,
                                        scalar2=None, op0=ALU.add)
            return t

        def load_bc(pool, name, dram_handle, off, n=None):
            n = D if n is None else n
            t = pool.tile([P, n], F32, name=name)
            nc.sync.dma_start(t[:], ap2(dram_handle, off, [[0, P], [1, n]]))
            return t

        def load_ada_bc(pool, name, off):
            """broadcast-row [P, D] load of an ada 1024-slice."""
            assert off % 512 == 0
            t = pool.tile([P, D], F32, name=name)
            for half in range(2):
                ch = ada_chunks[off // 512 + half]
                nc.sync.dma_start(t[:, half * 512:(half + 1) * 512],
                                  ap2(ch.tensor, 0, [[0, P], [1, 512]]))
            return t

        def ln_normalize(pool, xt):
            """token-major LN (no affine): (x - mean) * rsqrt(var + eps).
            Returns a BF16 normalized tile."""
            lp = pool.tile([P, 16], F32, tag="lnp")
            stats = lp[:, 0:12].rearrange("p (s f) -> p s f", f=6)
            xv = xt[:].rearrange("p (s f) -> p s f", f=512)
            for s in range(D // 512):
                nc.vector.bn_stats(stats[:, s, :], xv[:, s, :])
            mv = lp[:, 12:14]
            nc.vector.bn_aggr(mv, lp[:, 0:12].rearrange(
                "p (s f) -> p s f", f=6)[:, :D // 512, :])
            sd = lp[:, 14:15]
            nc.scalar.activation(sd, mv[:, 1:2], AF.Sqrt, bias=eps_t)
            rstd = lp[:, 15:16]
            nc.vector.reciprocal(rstd, sd)
            xn = pool.tile([P, D], BF16, tag="xn")
            nc.vector.tensor_scalar(xn[:], xt[:], scalar1=mv[:, 0:1],
                                    scalar2=rstd,
                                    op0=ALU.subtract, op1=ALU.mult)
            return xn

        # SBUF-resident intermediates. Released manually; allocation order is
        # chosen so releases pop in LIFO order per side:
        #   left: consts(ctx) < modg(ctx) < ores(til end of E)
        #         < kqres,vres(til end of D) < phase with-pools
        #   right: w2p(ctx) < res2(ctx)
        modg = ctx.enter_context(tc.tile_pool(name="modg", bufs=1))
        ores = tc.alloc_tile_pool(name="ores", bufs=1)
        oT = ores.tile([P, DC, NQ], BF16)
        kqres = tc.alloc_tile_pool(name="kqres", bufs=1)
        vres = tc.alloc_tile_pool(name="vres", bufs=1)
        if True:
            kT = kqres.tile([P, DC, NT], BF16)
            qT = kqres.tile([P, DC, NQ], BF16)
            V_aug = vres.tile([P, KT, NH * HC], BF16)
            with tc.tile_pool(name="hres_pool", bufs=1) as hres_pool, \
                 tc.tile_pool(name="mod1", bufs=1) as mod1, \
                 tc.tile_pool(name="ada_w", bufs=3) as awp, \
                 tc.tile_pool(name="ada_sb", bufs=3) as asb, \
                 tc.tile_pool(name="ln1", bufs=3) as lnp, \
                 tc.tile_pool(name="wkq", bufs=16) as wkq, \
                 tc.tile_pool(name="wv", bufs=9) as wvp, \
                 tc.tile_pool(name="ada_ps", bufs=1, space="PSUM") as aps, \
                 tc.tile_pool(name="tps", bufs=2, space="PSUM") as tps, \
                 tc.tile_pool(name="kqps", bufs=3, space="PSUM") as kqps, \
                 tc.tile_pool(name="vps", bufs=2, space="PSUM") as vps:
                hres = hres_pool.tile([P, DC, NT], BF16)
                # ======== Phase A: ada = cb @ W_ada + b_ada -> ada_chunks
                cT = asb.tile([P, DC], BF16)
                nc.sync.dma_start(cT[:], ap2(cb, 0, [[1, P], [P, DC]]))
                for j in range(6 * D // 512):
                    ps = aps.tile([1, 512], F32, tag="ps")
                    for kc in range(DC):
                        wt = awp.tile([P, 512], BF16, tag="w")
                        enga = nc.scalar if kc % 2 == 0 else nc.sync
                        enga.dma_start(
                            wt[:], ap2(W_ada, kc * P * 6 * D + j * 512,
                                       [[6 * D, P], [1, 512]]))
                        nc.tensor.matmul(ps[:], cT[:, kc:kc + 1], wt[:],
                                         start=(kc == 0), stop=(kc == DC - 1))
                    bt = asb.tile([1, 512], F32, tag="b")
                    nc.sync.dma_start(bt[:], ap2(b_ada, j * 512,
                                                 [[512, 1], [1, 512]]))
                    st = asb.tile([1, 512], F32, tag="s")
                    nc.vector.tensor_tensor(st[:], ps[:], bt[:], op=ALU.add)
                    nc.sync.dma_start(
                        ap2(ada_chunks[j].tensor, 0, [[512, 1], [1, 512]]),
                        st[:])

                # ======== Phase C-LN: LN1 + transpose + fused modulation
                s1_pp = load_pp(1 * D, plus1=True)
                sh1_pp = load_pp(0 * D)
                for t in range(KT):
                    xt = lnp.tile([P, D], F32, tag="x")
                    # SWDGE queue keeps x loads off the HWDGE rings that
                    # stream W_ada/weights at kernel start
                    nc.gpsimd.dma_start(xt[:], ap2(xb, t * P * D,
                                                   [[D, P], [1, D]]))
                    xn = ln_normalize(lnp, xt)
                    for dc in range(DC):
                        pt = tps.tile([P, P], BF16, tag="t")
                        nc.tensor.transpose(pt[:], xn[:, dc * P:(dc + 1) * P],
                                            ident[:])
                        if dc % 2 == 0:
                            nc.vector.tensor_scalar(
                                hres[:, dc, t * P:(t + 1) * P], pt[:],
                                scalar1=s1_pp[:, dc:dc + 1],
                                scalar2=sh1_pp[:, dc:dc + 1],
                                op0=ALU.mult, op1=ALU.add)
                        else:
                            nc.scalar.activation(
                                hres[:, dc, t * P:(t + 1) * P], pt[:],
                                AF.Identity,
                                scale=s1_pp[:, dc:dc + 1],
                                bias=sh1_pp[:, dc:dc + 1])

                # ======== Phase C-KQ: K^T, Q^T -> SBUF (kT/qT). Weights
                # loaded as [P, 512] spans (line-rate DMA), sliced per mc.
                for mcg in range(DC // 4):
                    for W_, b_pp, is_q in ((Wk, bk_pp, False),
                                           (Wq, bq_pp, True)):
                        ncols = NQ if is_q else NT
                        dst3 = qT if is_q else kT
                        wg = []
                        for kc in range(DC):
                            wt = wkq.tile([P, 512], BF16, tag="w")
                            eng = nc.scalar if kc % 2 == 0 else nc.sync
                            eng.dma_start(
                                wt[:], ap2(W_, kc * P * D + mcg * 512,
                                           [[D, P], [1, 512]]))
                            wg.append(wt)
                        for mci in range(4):
                            mc = mcg * 4 + mci
                            for ns in range(ncols // 512):
                                ps = kqps.tile([P, 512], F32, tag="ps")
                                for kc in range(DC):
                                    nc.tensor.matmul(
                                        ps[:],
                                        wg[kc][:, mci * P:(mci + 1) * P],
                                        hres[:, kc, ns * 512:(ns + 1) * 512],
                                        start=(kc == 0), stop=(kc == DC - 1))
                                nc.vector.tensor_scalar(
                                    dst3[:, mc, ns * 512:(ns + 1) * 512], ps[:],
                                    scalar1=b_pp[:, mc:mc + 1],
                                    scalar2=None, op0=ALU.add)

                # ======== Phase C-V: V_aug (token-major, per head + ones col)
                bv_bc = load_bc(mod1, "bv_bc", bv, 0)
                nc.scalar.copy(
                    V_aug[:].rearrange("p t (h c) -> p t h c",
                                       c=HC)[:, :, :, HD:HD + 1],
                    one_col.to_broadcast((P, KT, NH, 1)))
                for nh in range(D // 512):
                    wvt = []
                    for kc in range(DC):
                        wt = wvp.tile([P, 512], BF16, tag="wv")
                        eng = nc.scalar if kc % 2 == 0 else nc.sync
                        eng.dma_start(
                            wt[:], ap2(Wv, kc * P * D + nh * 512,
                                       [[D, P], [1, 512]]))
                        wvt.append(wt)
                    for t in range(KT):
                        ps = vps.tile([P, 512], F32, tag="ps")
                        for kc in range(DC):
                            nc.tensor.matmul(
                                ps[:], hres[:, kc, t * P:(t + 1) * P],
                                wvt[kc][:],
                                start=(kc == 0), stop=(kc == DC - 1))
                        dst = V_aug[:, t,
                                    nh * 8 * HC:(nh + 1) * 8 * HC].rearrange(
                            "p (h c) -> p h c", c=HC)[:, :, 0:HD]
                        nc.vector.tensor_tensor(
                            dst, ps[:].rearrange("p (h c) -> p h c", c=HD),
                            bv_bc[:, nh * 512:(nh + 1) * 512].rearrange(
                                "p (h c) -> p h c", c=HD),
                            op=ALU.add)

            # W2 prefetch: DMA engines are idle during attention, so stream
            # all of W2 into SBUF (right side) while phase D runs.
            w2p = ctx.enter_context(
                tc.tile_pool(name="w2p", bufs=1, side="right"))
            w2t = []
            for f in range(FC):
                wt = w2p.tile([P, D], BF16, tag=f"w2_{f}")
                eng = nc.sync if f % 2 == 0 else nc.scalar
                eng.dma_start(wt[:], ap2(W2, f * P * D, [[D, P], [1, D]]))
                w2t.append(wt)

            # ============ Phase D: attention per head
            with tc.tile_pool(name="expool", bufs=3) as expool, \
                 tc.tile_pool(name="rzp", bufs=2) as rzp, \
                 tc.tile_pool(name="sps", bufs=2, space="PSUM") as sps, \
                 tc.tile_pool(name="ops", bufs=2, space="PSUM") as ops:
                for h in range(NH):
                    hcc, hr = h // 2, (h % 2) * HD
                    po = ops.tile([HC, NQ], F32, tag="o")
                    for kt in range(KT):
                        pss = sps.tile([P, NQ], F32, tag="s")
                        for qs in range(NQ // 512):
                            nc.tensor.matmul(
                                pss[:, qs * 512:(qs + 1) * 512],
                                kT[hr:hr + HD, hcc, kt * P:(kt + 1) * P],
                                qT[hr:hr + HD, hcc, qs * 512:(qs + 1) * 512],
                                start=True, stop=True)
                        ex = expool.tile([P, NQ], BF16, tag="ex")
                        nc.scalar.activation(ex[:], pss[:], AF.Exp, scale=0.125)
                        for qs in range(NQ // 512):
                            nc.tensor.matmul(
                                po[:, qs * 512:(qs + 1) * 512],
                                V_aug[:, kt, h * HC:(h + 1) * HC],
                                ex[:, qs * 512:(qs + 1) * 512],
                                start=(kt == 0), stop=(kt == KT - 1))
                    rz = rzp.tile([1, NQ], F32, tag="rz")
                    nc.vector.reciprocal(rz[:], po[HD:HD + 1, :])
                    rzb = rzp.tile([HD, NQ], F32, tag="rzb")
                    nc.gpsimd.partition_broadcast(rzb[:], rz[:])
                    nc.vector.tensor_tensor(oT[hr:hr + HD, hcc, :],
                                            po[0:HD, :], rzb[:], op=ALU.mult)
            vres.release()
            kqres.release()

        # ============ Phase E: out-proj (gate1 folded into Wo) + residual
        # + LN2 (token-major x2 kept for the final residual)
        res2 = ctx.enter_context(tc.tile_pool(name="res2", bufs=1, side="right"))
        h2T = res2.tile([P, DC, NQ], BF16)
        x2 = res2.tile([P, QT, D], F32)
        g2_bc = load_ada_bc(modg, "g2_bc", 5 * D)
        with tc.tile_pool(name="mod2", bufs=1) as mod2, \
             tc.tile_pool(name="wo", bufs=1) as wop, \
             tc.tile_pool(name="ln2", bufs=3) as ln2p, \
             tc.tile_pool(name="aops", bufs=2, space="PSUM") as aops, \
             tc.tile_pool(name="tps2", bufs=4, space="PSUM") as tps2:
            g1_bc = load_ada_bc(mod2, "g1_bc", 2 * D)
            g1_bcb = mod2.tile([P, D], BF16, name="g1_bcb")
            nc.vector.tensor_copy(g1_bcb[:], g1_bc[:])
            s2_pp = load_pp(4 * D, plus1=True)
            sh2_pp = load_pp(3 * D)
            # cst_bc = g1*bo + g2*b2 (broadcast row; pre-folded into x2)
            cst_bc = mod2.tile([P, D], F32, name="cst_bc")
            tmp1 = ln2p.tile([P, D], F32, tag="tmp1", bufs=1)
            nc.sync.dma_start(tmp1[:], ap2(bo, 0, [[0, P], [1, D]]))
            nc.vector.tensor_tensor(cst_bc[:], g1_bc[:], tmp1[:], op=ALU.mult)
            nc.sync.dma_start(tmp1[:], ap2(b2, 0, [[0, P], [1, D]]))
            nc.vector.tensor_tensor(tmp1[:], tmp1[:], g2_bc[:], op=ALU.mult)
            nc.vector.tensor_tensor(cst_bc[:], cst_bc[:], tmp1[:], op=ALU.add)
            # Wo' rhs tiles (g1 pre-scaled), all resident, read once
            wot = {}
            for nh in range(D // 512):
                for oc in range(DC):
                    wt = wop.tile([P, 512], BF16, tag=f"w{nh}_{oc}")
                    eng = nc.scalar if oc % 2 == 0 else nc.sync
                    eng.dma_start(
                        wt[:], ap2(Wo, oc * P * D + nh * 512,
                                   [[D, P], [1, 512]]))
                    nc.vector.tensor_tensor(
                        wt[:], wt[:], g1_bcb[:, nh * 512:(nh + 1) * 512],
                        op=ALU.mult)
                    wot[(nh, oc)] = wt
            for t in range(QT):
                xt = ln2p.tile([P, D], F32, tag="x")
                nc.sync.dma_start(xt[:], ap2(xb, t * P * D, [[D, P], [1, D]]))
                # residual pre-add on the (otherwise idle) gpsimd engine
                nc.gpsimd.tensor_tensor(xt[:], xt[:], cst_bc[:], op=ALU.add)
                for nh in range(D // 512):
                    ps = aops.tile([P, 512], F32, tag="ps")
                    for oc in range(DC):
                        nc.tensor.matmul(ps[:], oT[:, oc, t * P:(t + 1) * P],
                                         wot[(nh, oc)][:],
                                         start=(oc == 0), stop=(oc == DC - 1))
                    sl = slice(nh * 512, (nh + 1) * 512)
                    nc.vector.tensor_tensor(x2[:, t, sl], xt[:, sl], ps[:],
                                            op=ALU.add)
                xn2 = ln_normalize(ln2p, x2[:, t, :])
                for dc in range(DC):
                    pt = tps2.tile([P, P], BF16, tag="t")
                    nc.tensor.transpose(pt[:], xn2[:, dc * P:(dc + 1) * P],
                                        ident[:])
                    # LN2 modulation all on ACT (DVE is the busy engine here)
                    nc.scalar.activation(
                        h2T[:, dc, t * P:(t + 1) * P], pt[:],
                        AF.Identity,
                        scale=s2_pp[:, dc:dc + 1],
                        bias=sh2_pp[:, dc:dc + 1])
        ores.release()

        # ============ Phase F: FFN. g = gelu(h2 @ W1 + b1) staged in SBUF
        # (feature-major); second matmul accumulates all of d_ff in PSUM with
        # token-major output; final residual+gate fused per token tile.
        with tc.tile_pool(name="w1p", bufs=2) as w1p, \
             tc.tile_pool(name="gres", bufs=1) as gres, \
             tc.tile_pool(name="fin", bufs=2) as finp, \
             tc.tile_pool(name="gps", bufs=2, space="PSUM") as gps, \
             tc.tile_pool(name="fps", bufs=2, space="PSUM") as fps:
            g = gres.tile([P, FC, NQ], BF16)
            for fb in range(FB):
                w1t = []
                for kc in range(DC):
                    wt = w1p.tile([P, FCB * P], BF16, tag=f"w1_{kc}")
                    eng1 = nc.scalar if kc % 2 == 0 else nc.sync
                    eng1.dma_start(
                        wt[:], ap2(W1, kc * P * DFF + fb * FCB * P,
                                   [[DFF, P], [1, FCB * P]]))
                    w1t.append(wt)
                for fc in range(FCB):
                    f = fb * FCB + fc
                    for qs in range(NQ // 512):
                        psg = gps.tile([P, 512], F32, tag="g")
                        for kc in range(DC):
                            nc.tensor.matmul(
                                psg[:], w1t[kc][:, fc * P:(fc + 1) * P],
                                h2T[:, kc, qs * 512:(qs + 1) * 512],
                                start=(kc == 0), stop=(kc == DC - 1))
                        if GELU_FUNC == "sigmoid_approx":
                            # CoreSim has no Gelu; x*sigmoid(1.702x) stand-in
                            xb1 = finp.tile([P, 512], F32, tag="xb1")
                            nc.vector.tensor_scalar(xb1[:], psg[:],
                                                    scalar1=b1_pp[:, f:f + 1],
                                                    scalar2=None, op0=ALU.add)
                            sg = finp.tile([P, 512], F32, tag="sg")
                            nc.scalar.activation(sg[:], xb1[:], AF.Sigmoid,
                                                 scale=1.702)
                            nc.vector.tensor_tensor(
                                g[:, f, qs * 512:(qs + 1) * 512],
                                xb1[:], sg[:], op=ALU.mult)
                        else:
                            nc.scalar.activation(
                                g[:, f, qs * 512:(qs + 1) * 512], psg[:],
                                GELU_FUNC, bias=b1_pp[:, f:f + 1])
            for t in range(QT):
                psf = fps.tile([P, D], F32, tag="f")
                for f in range(FC):
                    for nh in range(D // 512):
                        nc.tensor.matmul(
                            psf[:, nh * 512:(nh + 1) * 512],
                            g[:, f, t * P:(t + 1) * P],
                            w2t[f][:, nh * 512:(nh + 1) * 512],
                            start=(f == 0), stop=(f == FC - 1))
                o_t = finp.tile([P, D], F32, tag="o")
                nc.vector.tensor_tensor(o_t[:], psf[:], g2_bc[:], op=ALU.mult)
                nc.vector.tensor_tensor(o_t[:], o_t[:], x2[:, t, :], op=ALU.add)
                nc.sync.dma_start(ap2(out, t * P * D, [[D, P], [1, D]]), o_t[:])

    return {}


_COMPILED = None


def _get_compiled():
    global _COMPILED
    if _COMPILED is None:
        from concourse import bacc
        nc = bacc.Bacc("TRN2", target_bir_lowering=False, debug=False)
        build_dit(nc)
        nc.compile()
        _COMPILED = nc
    return _COMPILED


def _to_bf16(a):
    import ml_dtypes
    return np.asarray(a, np.float32).astype(ml_dtypes.bfloat16)


def make_in_maps(x, c, W_ada, b_ada, Wq, bq, Wk, bk, Wv, bv, Wo, bo,
                 W1, b1, W2, b2):
    x = np.ascontiguousarray(np.asarray(x, dtype=np.float32))
    shared = {
        "W_ada": _to_bf16(W_ada),
        "b_ada": np.asarray(b_ada, np.float32).reshape(1, -1),
        "Wq": _to_bf16(Wq), "bq": np.asarray(bq, np.float32).reshape(1, -1),
        "Wk": _to_bf16(Wk), "bk": np.asarray(bk, np.float32).reshape(1, -1),
        "Wv": _to_bf16(Wv), "bv": np.asarray(bv, np.float32).reshape(1, -1),
        "Wo": _to_bf16(Wo), "bo": np.asarray(bo, np.float32).reshape(1, -1),
        "W1": _to_bf16(W1), "b1": np.asarray(b1, np.float32).reshape(1, -1),
        "W2": _to_bf16(W2), "b2": np.asarray(b2, np.float32).reshape(1, -1),
    }
    cb = _to_bf16(np.asarray(c, np.float32))
    in_maps = []
    for core in range(8):
        b, s = core // 2, core % 2
        xb_ = np.roll(x[b], -1024 * s, axis=0) if s else x[b]
        m = dict(shared)
        m["xb"] = np.ascontiguousarray(xb_)
        m["cb"] = np.ascontiguousarray(cb[b:b + 1])
        in_maps.append(m)
    return in_maps


def kernel(x, c, W_ada, b_ada, Wq, bq, Wk, bk, Wv, bv, Wo, bo, W1, b1, W2, b2):
    from concourse import bass_utils
    nc = _get_compiled()
    B, N, D = x.shape
    assert (B, N, D) == (4, 2048, 1024)
    in_maps = make_in_maps(x, c, W_ada, b_ada, Wq, bq, Wk, bk, Wv, bv,
                           Wo, bo, W1, b1, W2, b2)

    last_err = None
    for _attempt in range(3):
        try:
            res = bass_utils.run_bass_kernel_spmd(nc, in_maps, core_ids=list(range(8)))
            break
        except Exception as e:  # transient NRT device errors; retry
            last_err = e
    else:
        raise last_err

    out = np.empty((4, 2048, 1024), np.float32)
    for core in range(8):
        b, s = core // 2, core % 2
        out[b, s * 1024:(s + 1) * 1024, :] = res.results[core]["out"]
    return out
